# revision 6
# baseline (speedup 1.0000x reference)
"""Cross-layer transcoder kernel for Trainium2 (8 NeuronCores, SPMD).

Math (from the reference):
    feats[l] = relu(x[l] @ W_enc[l].T + b_enc[l])          # [B, F] per layer
    recon[j] = sum_{i<=j} feats[i] @ W_dec[i, j] + b_dec[j] # [B, D] per layer

Sharding: the transcoder feature dim F=4096 is split across the 8 cores
(512 features each). Each core encodes its feature slice for all layers and
computes a partial reconstruction for every destination layer; the partials
are summed on the host (the gather/unshard step), where b_dec is also added.

Device layout notes:
  - All matmul operands are pre-transposed/packed on the host so every DMA is
    a contiguous [128, *] tile load and the PE contraction dim (d for encode,
    f for decode) lands on the partition axis.
  - Matmul inputs are cast to bf16 on the host; accumulation is fp32 in PSUM,
    the bias-add + relu runs in fp32 on the scalar engine. Measured end-to-end
    relative error vs the fp32 reference is ~3e-3.
  - Only the 36 upper-triangular (i, j) pairs of W_dec are ever transferred
    or computed.
"""

import os

import numpy as np
import ml_dtypes

L = 8          # n_layers
B = 1024       # n_pos
D = 768        # d_model
F = 4096       # d_transcoder
NCORES = 8
FL = F // NCORES   # features per core = 512
P = 128
KD = D // P        # 6  encode contraction chunks
MF = FL // P       # 4  feature chunks per core
MD = D // P        # 6  decode output chunks
NB = B // 512      # 2  position chunks of 512
PAIRS = [(i, j) for j in range(L) for i in range(j + 1)]
NPAIR = len(PAIRS)  # 36

BF16 = ml_dtypes.bfloat16

# Filled by the first kernel() call; reused afterwards.
_PROGRAM = None
# Stash of the most recent run's profiling results (test.py reads these).
LAST_EXEC_NS = None
LAST_RESULTS = None


def _build_program():
    import concourse.bacc as bacc
    import concourse.mybir as mybir
    import concourse.tile as tile

    nc = bacc.Bacc("TRN2", target_bir_lowering=False, debug=False)
    bf = mybir.dt.bfloat16
    f32 = mybir.dt.float32

    xT_d = nc.dram_tensor("xT", [L, KD, P, B], bf, kind="ExternalInput")
    wencT_d = nc.dram_tensor("wencT", [L, KD, P, FL], bf, kind="ExternalInput")
    benc_d = nc.dram_tensor("benc", [L, MF, P, 1], f32, kind="ExternalInput")
    wdec_d = nc.dram_tensor("wdec", [NPAIR, MF, P, D], bf, kind="ExternalInput")
    out_d = nc.dram_tensor("outT", [L, D, B], f32, kind="ExternalOutput")

    relu = mybir.ActivationFunctionType.Relu

    with tile.TileContext(nc) as tc:
        with (
            tc.tile_pool(name="feats", bufs=1) as feats_pool,
            tc.tile_pool(name="benc", bufs=1) as benc_pool,
            tc.tile_pool(name="xt", bufs=12) as xt_pool,
            tc.tile_pool(name="wenc", bufs=12) as wenc_pool,
            tc.tile_pool(name="wdec", bufs=44) as wdec_pool,
            tc.tile_pool(name="outs", bufs=8) as out_pool,
            tc.tile_pool(name="psum", bufs=8, space="PSUM") as psum_pool,
        ):
            feats = {}
            for j in range(L):
                # ---------- encode layer j into feats[(j, mf)] ----------
                xts = []
                wes = []
                for kd in range(KD):
                    xt = xt_pool.tile([P, B], bf, name="xt", tag="xt")
                    nc.gpsimd.dma_start(xt, xT_d[j, kd])
                    xts.append(xt)
                    we = wenc_pool.tile([P, FL], bf, name="we", tag="we")
                    nc.gpsimd.dma_start(we, wencT_d[j, kd])
                    wes.append(we)
                for mf in range(MF):
                    bt = benc_pool.tile([P, 1], f32, name=f"benc_{j}_{mf}")
                    nc.gpsimd.dma_start(bt, benc_d[j, mf])
                    ft = feats_pool.tile([P, B], bf, name=f"feat_{j}_{mf}")
                    feats[(j, mf)] = ft
                    for nb in range(NB):
                        ps = psum_pool.tile([P, 512], f32, name="ps", tag="psum")
                        for kd in range(KD):
                            nc.tensor.matmul(
                                ps,
                                lhsT=wes[kd][:, mf * P:(mf + 1) * P],
                                rhs=xts[kd][:, nb * 512:(nb + 1) * 512],
                                start=(kd == 0),
                                stop=(kd == KD - 1),
                            )
                        nc.scalar.activation(
                            ft[:, nb * 512:(nb + 1) * 512], ps, relu, bias=bt
                        )

                # ---------- decode destination layer j ----------
                nmm = (j + 1) * MF
                wts = {}
                for i in range(j + 1):
                    pidx = j * (j + 1) // 2 + i
                    for kf in range(MF):
                        wt = wdec_pool.tile([P, D], bf, name="wd", tag="wd")
                        nc.sync.dma_start(wt, wdec_d[pidx, kf])
                        wts[(i, kf)] = wt
                for md in range(MD):
                    for nb in range(NB):
                        ps = psum_pool.tile([P, 512], f32, name="ps", tag="psum")
                        c = 0
                        for i in range(j + 1):
                            for kf in range(MF):
                                nc.tensor.matmul(
                                    ps,
                                    lhsT=wts[(i, kf)][:, md * P:(md + 1) * P],
                                    rhs=feats[(i, kf)][:, nb * 512:(nb + 1) * 512],
                                    start=(c == 0),
                                    stop=(c == nmm - 1),
                                )
                                c += 1
                        ot = out_pool.tile([P, 512], f32, name="ot", tag="ot")
                        nc.vector.tensor_copy(ot, ps)
                        nc.scalar.dma_start(
                            out_d[j, md * P:(md + 1) * P, nb * 512:(nb + 1) * 512], ot
                        )

    nc.compile()
    return nc


def _prepare_inputs(x, W_enc, b_enc, W_dec):
    """Host-side shard + pack + cast. Returns in_maps for the 8 cores."""
    xT = np.ascontiguousarray(x.transpose(0, 2, 1)).astype(BF16).reshape(L, KD, P, B)
    in_maps = []
    for c in range(NCORES):
        s = slice(c * FL, (c + 1) * FL)
        wencT = (
            np.ascontiguousarray(W_enc[:, s, :].transpose(0, 2, 1))
            .astype(BF16)
            .reshape(L, KD, P, FL)
        )
        benc = np.ascontiguousarray(b_enc[:, s], dtype=np.float32).reshape(L, MF, P, 1)
        wdec = np.empty((NPAIR, MF, P, D), dtype=BF16)
        for pidx, (i, j) in enumerate(PAIRS):
            wdec[pidx] = W_dec[i, j, s, :].astype(BF16).reshape(MF, P, D)
        in_maps.append({"xT": xT, "wencT": wencT, "benc": benc, "wdec": wdec})
    return in_maps


def kernel(x, W_enc, b_enc, W_dec, b_dec):
    global _PROGRAM, LAST_EXEC_NS, LAST_RESULTS
    from concourse import bass_utils

    x = np.asarray(x)
    W_enc = np.asarray(W_enc)
    b_enc = np.asarray(b_enc)
    W_dec = np.asarray(W_dec)
    b_dec = np.asarray(b_dec)

    if _PROGRAM is None:
        _PROGRAM = _build_program()
    nc = _PROGRAM

    in_maps = _prepare_inputs(x, W_enc, b_enc, W_dec)

    trace = os.environ.get("KERNEL_TRACE", "0") == "1"
    res = bass_utils.run_bass_kernel_spmd(
        nc, in_maps, core_ids=list(range(NCORES)), trace=trace
    )
    LAST_EXEC_NS = res.exec_time_ns
    LAST_RESULTS = res

    acc = np.zeros((L, D, B), dtype=np.float32)
    for r in res.results:
        acc += np.asarray(r["outT"], dtype=np.float32)
    out = acc.transpose(0, 2, 1) + b_dec.astype(np.float32)[:, None, :]
    return np.ascontiguousarray(out, dtype=np.float32)


# revision 7
# speedup vs baseline: 1.0318x; 1.0318x over previous
"""Cross-layer transcoder kernel for Trainium2 (8 NeuronCores, SPMD).

Math (from the reference):
    feats[l] = relu(x[l] @ W_enc[l].T + b_enc[l])          # [B, F] per layer
    recon[j] = sum_{i<=j} feats[i] @ W_dec[i, j] + b_dec[j] # [B, D] per layer

Sharding: the transcoder feature dim F=4096 is split across the 8 cores
(512 features each). Each core encodes its feature slice for all layers and
computes a partial reconstruction for every destination layer; the partials
are summed on the host (the gather/unshard step), where b_dec is also added.

Device layout notes:
  - All matmul operands are pre-transposed/packed on the host so every DMA is
    a contiguous [128, *] tile load and the PE contraction dim (d for encode,
    f for decode) lands on the partition axis.
  - Matmul inputs are cast to bf16 on the host; accumulation is fp32 in PSUM,
    the bias-add + relu runs in fp32 on the scalar engine. Measured end-to-end
    relative error vs the fp32 reference is ~3e-3.
  - Only the 36 upper-triangular (i, j) pairs of W_dec are ever transferred
    or computed.
"""

import os

import numpy as np
import ml_dtypes

L = 8          # n_layers
B = 1024       # n_pos
D = 768        # d_model
F = 4096       # d_transcoder
NCORES = 8
FL = F // NCORES   # features per core = 512
P = 128
KD = D // P        # 6  encode contraction chunks
MF = FL // P       # 4  feature chunks per core
MD = D // P        # 6  decode output chunks
NB = B // 512      # 2  position chunks of 512
PAIRS = [(i, j) for j in range(L) for i in range(j + 1)]
NPAIR = len(PAIRS)  # 36

BF16 = ml_dtypes.bfloat16

# Filled by the first kernel() call; reused afterwards.
_PROGRAM = None
# Stash of the most recent run's profiling results (test.py reads these).
LAST_EXEC_NS = None
LAST_RESULTS = None


def _build_program():
    import concourse.bacc as bacc
    import concourse.mybir as mybir
    import concourse.tile as tile

    nc = bacc.Bacc("TRN2", target_bir_lowering=False, debug=False)
    bf = mybir.dt.bfloat16
    f32 = mybir.dt.float32

    xT_d = nc.dram_tensor("xT", [L, KD, P, B], bf, kind="ExternalInput")
    wencT_d = nc.dram_tensor("wencT", [L, KD, P, FL], bf, kind="ExternalInput")
    benc_d = nc.dram_tensor("benc", [L, MF, P, 1], f32, kind="ExternalInput")
    wdec_d = nc.dram_tensor("wdec", [NPAIR, MF, P, D], bf, kind="ExternalInput")
    out_d = nc.dram_tensor("outT", [L, D, B], f32, kind="ExternalOutput")

    relu = mybir.ActivationFunctionType.Relu

    with tile.TileContext(nc) as tc:
        with (
            tc.tile_pool(name="feats", bufs=1) as feats_pool,
            tc.tile_pool(name="benc", bufs=1) as benc_pool,
            tc.tile_pool(name="xt", bufs=12) as xt_pool,
            tc.tile_pool(name="wenc", bufs=12) as wenc_pool,
            tc.tile_pool(name="wdec", bufs=44) as wdec_pool,
            tc.tile_pool(name="outs", bufs=8) as out_pool,
            tc.tile_pool(name="psum", bufs=8, space="PSUM") as psum_pool,
        ):
            feats = {}
            for j in range(L):
                # ---------- encode layer j into feats[(j, mf)] ----------
                xts = []
                wes = []
                for kd in range(KD):
                    xt = xt_pool.tile([P, B], bf, name="xt", tag="xt")
                    nc.sync.dma_start(xt, xT_d[j, kd])
                    xts.append(xt)
                    we = wenc_pool.tile([P, FL], bf, name="we", tag="we")
                    nc.sync.dma_start(we, wencT_d[j, kd])
                    wes.append(we)
                for mf in range(MF):
                    bt = benc_pool.tile([P, 1], f32, name=f"benc_{j}_{mf}")
                    nc.scalar.dma_start(bt, benc_d[j, mf])
                    ft = feats_pool.tile([P, B], bf, name=f"feat_{j}_{mf}")
                    feats[(j, mf)] = ft
                    for nb in range(NB):
                        ps = psum_pool.tile([P, 512], f32, name="ps", tag="psum")
                        for kd in range(KD):
                            nc.tensor.matmul(
                                ps,
                                lhsT=wes[kd][:, mf * P:(mf + 1) * P],
                                rhs=xts[kd][:, nb * 512:(nb + 1) * 512],
                                start=(kd == 0),
                                stop=(kd == KD - 1),
                            )
                        nc.scalar.activation(
                            ft[:, nb * 512:(nb + 1) * 512], ps, relu, bias=bt
                        )

                # ---------- decode destination layer j ----------
                nmm = (j + 1) * MF
                wts = {}
                for i in range(j + 1):
                    pidx = j * (j + 1) // 2 + i
                    for kf in range(MF):
                        wt = wdec_pool.tile([P, D], bf, name="wd", tag="wd")
                        nc.sync.dma_start(wt, wdec_d[pidx, kf])
                        wts[(i, kf)] = wt
                for md in range(MD):
                    for nb in range(NB):
                        ps = psum_pool.tile([P, 512], f32, name="ps", tag="psum")
                        c = 0
                        for i in range(j + 1):
                            for kf in range(MF):
                                nc.tensor.matmul(
                                    ps,
                                    lhsT=wts[(i, kf)][:, md * P:(md + 1) * P],
                                    rhs=feats[(i, kf)][:, nb * 512:(nb + 1) * 512],
                                    start=(c == 0),
                                    stop=(c == nmm - 1),
                                )
                                c += 1
                        ot = out_pool.tile([P, 512], f32, name="ot", tag="ot")
                        nc.vector.tensor_copy(ot, ps)
                        nc.scalar.dma_start(
                            out_d[j, md * P:(md + 1) * P, nb * 512:(nb + 1) * 512], ot
                        )

    nc.compile()
    return nc


def _prepare_inputs(x, W_enc, b_enc, W_dec):
    """Host-side shard + pack + cast. Returns in_maps for the 8 cores."""
    xT = np.ascontiguousarray(x.transpose(0, 2, 1)).astype(BF16).reshape(L, KD, P, B)
    in_maps = []
    for c in range(NCORES):
        s = slice(c * FL, (c + 1) * FL)
        wencT = (
            np.ascontiguousarray(W_enc[:, s, :].transpose(0, 2, 1))
            .astype(BF16)
            .reshape(L, KD, P, FL)
        )
        benc = np.ascontiguousarray(b_enc[:, s], dtype=np.float32).reshape(L, MF, P, 1)
        wdec = np.empty((NPAIR, MF, P, D), dtype=BF16)
        for pidx, (i, j) in enumerate(PAIRS):
            wdec[pidx] = W_dec[i, j, s, :].astype(BF16).reshape(MF, P, D)
        in_maps.append({"xT": xT, "wencT": wencT, "benc": benc, "wdec": wdec})
    return in_maps


def kernel(x, W_enc, b_enc, W_dec, b_dec):
    global _PROGRAM, LAST_EXEC_NS, LAST_RESULTS
    from concourse import bass_utils

    x = np.asarray(x)
    W_enc = np.asarray(W_enc)
    b_enc = np.asarray(b_enc)
    W_dec = np.asarray(W_dec)
    b_dec = np.asarray(b_dec)

    if _PROGRAM is None:
        _PROGRAM = _build_program()
    nc = _PROGRAM

    in_maps = _prepare_inputs(x, W_enc, b_enc, W_dec)

    trace = os.environ.get("KERNEL_TRACE", "0") == "1"
    res = bass_utils.run_bass_kernel_spmd(
        nc, in_maps, core_ids=list(range(NCORES)), trace=trace
    )
    LAST_EXEC_NS = res.exec_time_ns
    LAST_RESULTS = res

    acc = np.zeros((L, D, B), dtype=np.float32)
    for r in res.results:
        acc += np.asarray(r["outT"], dtype=np.float32)
    out = acc.transpose(0, 2, 1) + b_dec.astype(np.float32)[:, None, :]
    return np.ascontiguousarray(out, dtype=np.float32)
